# revision 8
# baseline (speedup 1.0000x reference)
"""GC-LSTM (graph-conv LSTM) Trainium2 kernel.

Model (per batch b, timestep t):
    msg  = relu([x_t, h] @ Wg + bg)          # [N, H]
    agg  = adj @ msg                         # graph aggregation over stations
    gates= agg @ Wl + bl                     # [N, 4H]
    c    = sig(f)*c + sig(i)*tanh(g)
    h    = sig(o)*tanh(c)
final: out = relu(h[:, 0, :] @ Wr1 + br1) @ Wr2 + br2     # [B, 1]

Sharding: data-parallel over B=32 across 8 cores (B_local=4). adj and
weights replicated. h/c stay on-core through the whole T=168 scan.

Per-core layouts (b = 2*pair + q, q = parity, band(q) = q*64):
  SL (station-layout): partition = station-within-ktile.
    msg_sb   [128, (k,b,h)]  col = k*256 + b*64 + h
    adjT_sb  [128, (k,m)]    adjT_sb[p, k*512+m] = adj[m, k*128+p]
  PP (parity-paired feature-layout): partition = (q, h), free = (pair, m).
    hT2, cT2 [128, 1024]
    gates psum per pair: [128, (gate,m)] with gate order (i, f, o, g)
All elementwise ops are partition-aligned [128, *] in PP layout.
"""

import os
import sys
import types
from contextlib import ExitStack

import numpy as np

import concourse.bass as bass
import concourse.mybir as mybir
import concourse.tile as tile
from concourse import bacc
from concourse.bass_utils import run_bass_kernel_spmd


def _ensure_ntff_hook():
    """Register the axon NTFF profile hook if the image's antenv lacks it."""
    try:
        from antenv import axon_hooks  # noqa: F401
        return
    except ImportError:
        pass
    hook = None
    try:
        import importlib
        tb = importlib.import_module("trn_agent_boot.trn_boot")
        hook = tb._ntff_profile_via_ctypes("/opt/axon/libaxon_pjrt.so")
    except Exception:
        hook = None
    mod = types.ModuleType("antenv.axon_hooks")
    mod._hook = hook
    mod.set_axon_ntff_profile_hook = lambda h: setattr(mod, "_hook", h)
    mod.get_axon_ntff_profile_hook = lambda: mod._hook
    import antenv
    sys.modules["antenv.axon_hooks"] = mod
    antenv.axon_hooks = mod

AF = mybir.ActivationFunctionType
DT = mybir.dt

# Problem constants (hardcoded per contract)
B, N, T, F = 32, 512, 168, 8
H = 64
NCORES = 8
BL = B // NCORES          # 4 batches per core
NPAIR = BL // 2           # 2 pairs
KT = N // 128             # 4 station k-tiles
COLS = BL * N // 4        # 2048 / ... free width of SL tensors per 64h: BL*N/?  (unused)
BN = BL * N               # 2048


def build_program(t_steps: int = T, dt_data=DT.float32, dt_c=DT.float32,
                  n_devices: int = NCORES):
    """Build the SPMD Bass program for one core (replicated on all cores)."""
    nc = bacc.Bacc("TRN2", target_bir_lowering=False, debug=False,
                   num_devices=n_devices)

    fp32 = DT.float32

    # DRAM I/O (per-core values supplied via in_maps)
    xT_d = nc.dram_tensor("xT", [F + 1, t_steps, BN], dt_data, kind="ExternalInput").ap()
    adjT_d = nc.dram_tensor("adjT", [128, KT * N], dt_data, kind="ExternalInput").ap()
    wgx_d = nc.dram_tensor("wgx", [F + 1, H], dt_data, kind="ExternalInput").ap()
    wgh2_d = nc.dram_tensor("wgh2", [128, H], dt_data, kind="ExternalInput").ap()
    wl2_d = nc.dram_tensor("wl2", [128, 4 * H], dt_data, kind="ExternalInput").ap()
    wr12_d = nc.dram_tensor("wr12", [128, H // 2], fp32, kind="ExternalInput").ap()
    br1_d = nc.dram_tensor("br1", [H // 2, 1], fp32, kind="ExternalInput").ap()
    wr2_d = nc.dram_tensor("wr2", [H // 2, 1], fp32, kind="ExternalInput").ap()
    br2_d = nc.dram_tensor("br2", [1, 1], fp32, kind="ExternalInput").ap()
    out_d = nc.dram_tensor("out", [1, BL], fp32, kind="ExternalOutput").ap()

    with tile.TileContext(nc) as tc, ExitStack() as ctx:
        const_pool = ctx.enter_context(tc.tile_pool(name="const", bufs=1))
        state_pool = ctx.enter_context(tc.tile_pool(name="state", bufs=1))
        xc_pool = ctx.enter_context(tc.tile_pool(name="xc", bufs=3))
        msg_pool = ctx.enter_context(tc.tile_pool(name="msg", bufs=2))
        aggs_pool = ctx.enter_context(tc.tile_pool(name="aggs", bufs=4))
        gact_pool = ctx.enter_context(tc.tile_pool(name="gact", bufs=3))
        tmp_pool = ctx.enter_context(tc.tile_pool(name="tmp", bufs=4))

        pp_msg = ctx.enter_context(tc.tile_pool(name="pp_msg", bufs=1, space="PSUM"))
        pp_agg = ctx.enter_context(tc.tile_pool(name="pp_agg", bufs=2, space="PSUM"))
        pp_gate = ctx.enter_context(tc.tile_pool(name="pp_gate", bufs=1, space="PSUM"))

        # ---- constants ----
        adjT_sb = const_pool.tile([128, KT * N], dt_data)
        nc.sync.dma_start(adjT_sb[:], adjT_d[:])
        wgx_sb = const_pool.tile([F + 1, H], dt_data)
        nc.sync.dma_start(wgx_sb[:], wgx_d[:])
        wgh2_sb = const_pool.tile([128, H], dt_data)
        nc.sync.dma_start(wgh2_sb[:], wgh2_d[:])
        wl2_sb = const_pool.tile([128, 4 * H], dt_data)
        nc.sync.dma_start(wl2_sb[:], wl2_d[:])
        wr12_sb = const_pool.tile([128, H // 2], fp32)
        nc.sync.dma_start(wr12_sb[:], wr12_d[:])
        br1_sb = const_pool.tile([H // 2, 1], fp32)
        nc.sync.dma_start(br1_sb[:], br1_d[:])
        wr2_sb = const_pool.tile([H // 2, 1], fp32)
        nc.sync.dma_start(wr2_sb[:], wr2_d[:])
        br2_sb = const_pool.tile([1, 1], fp32)
        nc.sync.dma_start(br2_sb[:], br2_d[:])

        # ---- persistent state (PP layout), zero-init ----
        hT2 = state_pool.tile([128, NPAIR * N], dt_data)
        cT2 = state_pool.tile([128, NPAIR * N], dt_c)
        nc.gpsimd.memset(hT2[:], 0.0)
        nc.gpsimd.memset(cT2[:], 0.0)

        for t in range(t_steps):
            # x_t^T (+ ones row baked by host): [9, 2048]
            xc = xc_pool.tile([F + 1, BN], dt_data)
            nc.sync.dma_start(xc[:], xT_d[:, t, :])

            # ---- M1: msg = relu(x W_gx + bg + h W_gh), station-layout ----
            msg_ps = pp_msg.tile([128, BL * H * KT], fp32)  # [128, 1024]
            for r in range(BL * KT):           # row-tile r = b*KT + k
                b, k = divmod(r, KT)
                pair, q = divmod(b, 2)
                nc.tensor.matmul(
                    msg_ps[:, r * H:(r + 1) * H],
                    xc[:, r * 128:(r + 1) * 128],
                    wgx_sb[:],
                    start=True, stop=False,
                )
                nc.tensor.matmul(
                    msg_ps[:, r * H:(r + 1) * H],
                    hT2[q * H:(q + 1) * H,
                        pair * N + k * 128: pair * N + (k + 1) * 128],
                    wgh2_sb[q * H:(q + 1) * H, :],
                    start=False, stop=True,
                )

            # relu + permute (b,k,h) -> (k,b,h), PSUM -> SBUF
            msg_sb = msg_pool.tile([128, BL * H * KT], dt_data)
            nc.scalar.activation(
                msg_sb[:].rearrange("p (k b h) -> p b k h", k=KT, b=BL, h=H),
                msg_ps[:].rearrange("p (b k h) -> p b k h", b=BL, k=KT, h=H),
                AF.Relu,
            )

            for pair in range(NPAIR):
                # ---- M2: aggT_pair = msg_pair^T @ adjT  (PP out) ----
                agg_ps = pp_agg.tile([128, N], fp32)
                for k in range(KT):
                    nc.tensor.matmul(
                        agg_ps[:],
                        msg_sb[:, k * (BL * H) + pair * 128:
                               k * (BL * H) + (pair + 1) * 128],
                        adjT_sb[:, k * N:(k + 1) * N],
                        start=(k == 0), stop=(k == KT - 1),
                    )
                agg_sb = aggs_pool.tile([128, N], dt_data)
                nc.vector.tensor_copy(agg_sb[:], agg_ps[:])

                # ---- M3: gates (i,f,o,g) for this pair ----
                gate_ps = pp_gate.tile([128, 4 * N], fp32)  # [128, (gate, m)]
                for g in range(4):
                    for q in range(2):
                        nc.tensor.matmul(
                            gate_ps[q * H:(q + 1) * H, g * N:(g + 1) * N],
                            wl2_sb[q * H:(q + 1) * H, g * H:(g + 1) * H],
                            agg_sb[q * H:(q + 1) * H, :],
                            start=True, stop=True,
                            tile_position=(q * H, q * H),
                        )

                # ---- activations ----
                sig = gact_pool.tile([128, 3 * N], dt_data, tag="sig")
                nc.scalar.activation(sig[:], gate_ps[:, 0:3 * N], AF.Sigmoid)
                tng = gact_pool.tile([128, N], dt_data, tag="tng")
                nc.scalar.activation(tng[:], gate_ps[:, 3 * N:4 * N], AF.Tanh)

                # ---- LSTM state update (all [128, 512], PP-aligned) ----
                c_sl = cT2[:, pair * N:(pair + 1) * N]
                t1 = tmp_pool.tile([128, N], dt_c, tag="t1")
                nc.vector.tensor_mul(t1[:], sig[:, N:2 * N], c_sl)
                t2 = tmp_pool.tile([128, N], dt_c, tag="t2")
                nc.vector.tensor_mul(t2[:], sig[:, 0:N], tng[:])
                nc.vector.tensor_add(c_sl, t1[:], t2[:])
                tc_ = tmp_pool.tile([128, N], dt_data, tag="tc")
                nc.scalar.activation(tc_[:], c_sl, AF.Tanh)
                nc.vector.tensor_mul(hT2[:, pair * N:(pair + 1) * N],
                                     sig[:, 2 * N:3 * N], tc_[:])

        # ---- readout head: feat = h[:, station 0, :] ----
        r1_ps = pp_agg.tile([H // 2, BL], fp32, tag="agg_ps")
        for b in range(BL):
            pair, q = divmod(b, 2)
            nc.tensor.matmul(
                r1_ps[:, b:b + 1],
                wr12_sb[q * H:(q + 1) * H, :],
                hT2[q * H:(q + 1) * H, pair * N:pair * N + 1],
                start=True, stop=True,
            )
        r1_sb = tmp_pool.tile([H // 2, BL], fp32, tag="r1")
        nc.scalar.activation(r1_sb[:], r1_ps[:], AF.Relu, bias=br1_sb[:])
        r2_ps = pp_agg.tile([1, BL], fp32, tag="agg_ps")
        nc.tensor.matmul(r2_ps[:], wr2_sb[:], r1_sb[:], start=True, stop=True)
        out_sb = tmp_pool.tile([1, BL], fp32, tag="out")
        nc.scalar.activation(out_sb[:], r2_ps[:], AF.Identity, bias=br2_sb[:])
        nc.sync.dma_start(out_d[:], out_sb[:])

    nc.compile()
    return nc


def prep_inputs(pollution_seq, adj, Wg, bg, Wl, bl, Wr1, br1, Wr2, br2,
                t_steps: int = T, np_dt=np.float32):
    """Host-side prep: shard + relayout. Returns per-core in_maps list."""
    assert np.allclose(bl, 0.0), "kernel folds bl only for bl==0"
    f32 = np.float32

    adjT = np.ascontiguousarray(adj.T).astype(f32)          # [n, m]
    adjT_tiled = np.ascontiguousarray(
        adjT.reshape(KT, 128, N).transpose(1, 0, 2).reshape(128, KT * N)
    ).astype(np_dt)

    wgx = np.concatenate([Wg[:F], bg[None, :]], axis=0).astype(np_dt)   # [9, 64]
    wgh = Wg[F:].astype(f32)                                            # [64, 64]
    wgh2 = np.concatenate([wgh, wgh], axis=0).astype(np_dt)             # [128, 64]
    # gate order (i, f, g, o) -> (i, f, o, g)
    Wl_r = np.concatenate(
        [Wl[:, 0:H], Wl[:, H:2 * H], Wl[:, 3 * H:4 * H], Wl[:, 2 * H:3 * H]],
        axis=1).astype(f32)
    wl2 = np.concatenate([Wl_r, Wl_r], axis=0).astype(np_dt)            # [128, 256]
    wr12 = np.concatenate([Wr1, Wr1], axis=0).astype(f32)               # [128, 32]

    common = dict(
        adjT=adjT_tiled, wgx=wgx, wgh2=wgh2, wl2=wl2, wr12=wr12,
        br1=br1.reshape(H // 2, 1).astype(f32),
        wr2=Wr2.reshape(H // 2, 1).astype(f32),
        br2=br2.reshape(1, 1).astype(f32),
    )

    in_maps = []
    for i in range(NCORES):
        xc = pollution_seq[i * BL:(i + 1) * BL, :, :t_steps, :]   # [4, 512, t, 8]
        xT = np.ascontiguousarray(xc.transpose(3, 2, 0, 1))       # [8, t, 4, 512]
        xT = xT.reshape(F, t_steps, BN)
        ones = np.ones((1, t_steps, BN), dtype=f32)
        xT9 = np.concatenate([xT.astype(f32), ones], axis=0).astype(np_dt)
        m = dict(common)
        m["xT"] = np.ascontiguousarray(xT9)
        in_maps.append(m)
    return in_maps


_NC_CACHE = {}


def _get_program(t_steps, dt_data, dt_c):
    key = (t_steps, dt_data, dt_c)
    if key not in _NC_CACHE:
        _NC_CACHE[key] = build_program(t_steps, dt_data, dt_c)
    return _NC_CACHE[key]


def kernel(pollution_seq, adj, Wg, bg, Wl, bl, Wr1, br1, Wr2, br2,
           trace=False):
    if trace:
        _ensure_ntff_hook()
    dt_data, dt_c, np_dt = DT.float32, DT.float32, np.float32
    nc = _get_program(T, dt_data, dt_c)
    in_maps = prep_inputs(pollution_seq, adj, Wg, bg, Wl, bl, Wr1, br1,
                          Wr2, br2, T, np_dt)
    res = run_bass_kernel_spmd(nc, in_maps, list(range(NCORES)), trace=trace)
    outs = [res.results[i]["out"].reshape(1, BL) for i in range(NCORES)]
    full = np.concatenate([o.T for o in outs], axis=0).astype(np.float32)  # [32,1]
    if trace:
        kernel.last_exec_time_ns = res.exec_time_ns
        kernel.last_results = res
    return full


# revision 10
# speedup vs baseline: 1.9504x; 1.9504x over previous
"""GC-LSTM (graph-conv LSTM) Trainium2 kernel.

Model (per batch b, timestep t):
    msg  = relu([x_t, h] @ Wg + bg)          # [N, H]
    agg  = adj @ msg                         # graph aggregation over stations
    gates= agg @ Wl + bl                     # [N, 4H]
    c    = sig(f)*c + sig(i)*tanh(g)
    h    = sig(o)*tanh(c)
final: out = relu(h[:, 0, :] @ Wr1 + br1) @ Wr2 + br2     # [B, 1]

Sharding: data-parallel over B=32 across 8 cores (B_local=4). adj and
weights replicated. h/c stay on-core through the whole T=168 scan.

Per-core layouts (b = 2*pair + q, q = parity, band(q) = q*64):
  SL (station-layout): partition = station-within-ktile.
    msg_sb   [128, (k,b,h)]  col = k*256 + b*64 + h
    adjT_sb  [128, (k,m)]    adjT_sb[p, k*512+m] = adj[m, k*128+p]
  PP (parity-paired feature-layout): partition = (q, h), free = (pair, m).
    hT2, cT2 [128, 1024]
    gates psum per pair: [128, (gate,m)] with gate order (i, f, o, g)
All elementwise ops are partition-aligned [128, *] in PP layout.
"""

import os
import sys
import types
from contextlib import ExitStack

import numpy as np

import concourse.bass as bass
import concourse.mybir as mybir
import concourse.tile as tile
from concourse import bacc
from concourse.bass_utils import run_bass_kernel_spmd


def _ensure_ntff_hook():
    """Register the axon NTFF profile hook if the image's antenv lacks it."""
    try:
        from antenv import axon_hooks  # noqa: F401
        return
    except ImportError:
        pass
    hook = None
    try:
        import importlib
        tb = importlib.import_module("trn_agent_boot.trn_boot")
        hook = tb._ntff_profile_via_ctypes("/opt/axon/libaxon_pjrt.so")
    except Exception:
        hook = None
    mod = types.ModuleType("antenv.axon_hooks")
    mod._hook = hook
    mod.set_axon_ntff_profile_hook = lambda h: setattr(mod, "_hook", h)
    mod.get_axon_ntff_profile_hook = lambda: mod._hook
    import antenv
    sys.modules["antenv.axon_hooks"] = mod
    antenv.axon_hooks = mod

AF = mybir.ActivationFunctionType
DT = mybir.dt

# Problem constants (hardcoded per contract)
B, N, T, F = 32, 512, 168, 8
H = 64
NCORES = 8
BL = B // NCORES          # 4 batches per core
NPAIR = BL // 2           # 2 pairs
KT = N // 128             # 4 station k-tiles
COLS = BL * N // 4        # 2048 / ... free width of SL tensors per 64h: BL*N/?  (unused)
BN = BL * N               # 2048


def build_program(t_steps: int = T, dt_data=DT.float32, dt_c=DT.float32,
                  n_devices: int = NCORES):
    """Build the SPMD Bass program for one core (replicated on all cores)."""
    nc = bacc.Bacc("TRN2", target_bir_lowering=False, debug=False,
                   num_devices=n_devices)

    fp32 = DT.float32

    # DRAM I/O (per-core values supplied via in_maps)
    xT_d = nc.dram_tensor("xT", [F + 1, t_steps, BN], dt_data, kind="ExternalInput").ap()
    adjT_d = nc.dram_tensor("adjT", [128, KT * N], dt_data, kind="ExternalInput").ap()
    wgx_d = nc.dram_tensor("wgx", [F + 1, H], dt_data, kind="ExternalInput").ap()
    wgh2_d = nc.dram_tensor("wgh2", [128, H], dt_data, kind="ExternalInput").ap()
    wl2_d = nc.dram_tensor("wl2", [128, 4 * H], dt_data, kind="ExternalInput").ap()
    wr12_d = nc.dram_tensor("wr12", [128, H // 2], dt_data, kind="ExternalInput").ap()
    br1_d = nc.dram_tensor("br1", [H // 2, 1], fp32, kind="ExternalInput").ap()
    wr2_d = nc.dram_tensor("wr2", [H // 2, 1], dt_data, kind="ExternalInput").ap()
    br2_d = nc.dram_tensor("br2", [1, 1], fp32, kind="ExternalInput").ap()
    out_d = nc.dram_tensor("out", [1, BL], fp32, kind="ExternalOutput").ap()

    with tile.TileContext(nc) as tc, ExitStack() as ctx:
        const_pool = ctx.enter_context(tc.tile_pool(name="const", bufs=1))
        state_pool = ctx.enter_context(tc.tile_pool(name="state", bufs=1))
        xc_pool = ctx.enter_context(tc.tile_pool(name="xc", bufs=3))
        msg_pool = ctx.enter_context(tc.tile_pool(name="msg", bufs=2))
        aggs_pool = ctx.enter_context(tc.tile_pool(name="aggs", bufs=4))
        gact_pool = ctx.enter_context(tc.tile_pool(name="gact", bufs=3))
        tmp_pool = ctx.enter_context(tc.tile_pool(name="tmp", bufs=4))

        pp_msg = ctx.enter_context(tc.tile_pool(name="pp_msg", bufs=1, space="PSUM"))
        pp_agg = ctx.enter_context(tc.tile_pool(name="pp_agg", bufs=2, space="PSUM"))
        pp_gate = ctx.enter_context(tc.tile_pool(name="pp_gate", bufs=1, space="PSUM"))

        # ---- constants ----
        adjT_sb = const_pool.tile([128, KT * N], dt_data)
        nc.sync.dma_start(adjT_sb[:], adjT_d[:])
        wgx_sb = const_pool.tile([F + 1, H], dt_data)
        nc.sync.dma_start(wgx_sb[:], wgx_d[:])
        wgh2_sb = const_pool.tile([128, H], dt_data)
        nc.sync.dma_start(wgh2_sb[:], wgh2_d[:])
        wl2_sb = const_pool.tile([128, 4 * H], dt_data)
        nc.sync.dma_start(wl2_sb[:], wl2_d[:])
        wr12_sb = const_pool.tile([128, H // 2], dt_data)
        nc.sync.dma_start(wr12_sb[:], wr12_d[:])
        br1_sb = const_pool.tile([H // 2, 1], fp32)
        nc.sync.dma_start(br1_sb[:], br1_d[:])
        wr2_sb = const_pool.tile([H // 2, 1], dt_data)
        nc.sync.dma_start(wr2_sb[:], wr2_d[:])
        br2_sb = const_pool.tile([1, 1], fp32)
        nc.sync.dma_start(br2_sb[:], br2_d[:])

        # ---- persistent state (PP layout), zero-init ----
        hT2 = state_pool.tile([128, NPAIR * N], dt_data)
        cT2 = state_pool.tile([128, NPAIR * N], dt_c)
        nc.gpsimd.memset(hT2[:], 0.0)
        nc.gpsimd.memset(cT2[:], 0.0)

        for t in range(t_steps):
            # x_t^T (+ ones row baked by host): [9, 2048]
            xc = xc_pool.tile([F + 1, BN], dt_data)
            nc.sync.dma_start(xc[:], xT_d[:, t, :])

            # ---- M1: msg = relu(x W_gx + bg + h W_gh), station-layout ----
            msg_ps = pp_msg.tile([128, BL * H * KT], fp32)  # [128, 1024]
            for r in range(BL * KT):           # row-tile r = b*KT + k
                b, k = divmod(r, KT)
                pair, q = divmod(b, 2)
                nc.tensor.matmul(
                    msg_ps[:, r * H:(r + 1) * H],
                    xc[:, r * 128:(r + 1) * 128],
                    wgx_sb[:],
                    start=True, stop=False,
                )
                nc.tensor.matmul(
                    msg_ps[:, r * H:(r + 1) * H],
                    hT2[q * H:(q + 1) * H,
                        pair * N + k * 128: pair * N + (k + 1) * 128],
                    wgh2_sb[q * H:(q + 1) * H, :],
                    start=False, stop=True,
                )

            # relu + permute (b,k,h) -> (k,b,h), PSUM -> SBUF
            msg_sb = msg_pool.tile([128, BL * H * KT], dt_data)
            nc.scalar.activation(
                msg_sb[:].rearrange("p (k b h) -> p b k h", k=KT, b=BL, h=H),
                msg_ps[:].rearrange("p (b k h) -> p b k h", b=BL, k=KT, h=H),
                AF.Relu,
            )

            for pair in range(NPAIR):
                # ---- M2: aggT_pair = msg_pair^T @ adjT  (PP out) ----
                agg_ps = pp_agg.tile([128, N], fp32)
                for k in range(KT):
                    nc.tensor.matmul(
                        agg_ps[:],
                        msg_sb[:, k * (BL * H) + pair * 128:
                               k * (BL * H) + (pair + 1) * 128],
                        adjT_sb[:, k * N:(k + 1) * N],
                        start=(k == 0), stop=(k == KT - 1),
                    )
                agg_sb = aggs_pool.tile([128, N], dt_data)
                nc.vector.tensor_copy(agg_sb[:], agg_ps[:])

                # ---- M3: gates (i,f,o,g) for this pair ----
                gate_ps = pp_gate.tile([128, 4 * N], fp32)  # [128, (gate, m)]
                for g in range(4):
                    for q in range(2):
                        nc.tensor.matmul(
                            gate_ps[q * H:(q + 1) * H, g * N:(g + 1) * N],
                            wl2_sb[q * H:(q + 1) * H, g * H:(g + 1) * H],
                            agg_sb[q * H:(q + 1) * H, :],
                            start=True, stop=True,
                            tile_position=(q * H, q * H),
                        )

                # ---- activations ----
                sig = gact_pool.tile([128, 3 * N], dt_data, tag="sig")
                nc.scalar.activation(sig[:], gate_ps[:, 0:3 * N], AF.Sigmoid)
                tng = gact_pool.tile([128, N], dt_data, tag="tng")
                nc.scalar.activation(tng[:], gate_ps[:, 3 * N:4 * N], AF.Tanh)

                # ---- LSTM state update (all [128, 512], PP-aligned) ----
                c_sl = cT2[:, pair * N:(pair + 1) * N]
                t1 = tmp_pool.tile([128, N], dt_c, tag="t1")
                nc.vector.tensor_mul(t1[:], sig[:, N:2 * N], c_sl)
                t2 = tmp_pool.tile([128, N], dt_c, tag="t2")
                nc.vector.tensor_mul(t2[:], sig[:, 0:N], tng[:])
                nc.vector.tensor_add(c_sl, t1[:], t2[:])
                tc_ = tmp_pool.tile([128, N], dt_data, tag="tc")
                nc.scalar.activation(tc_[:], c_sl, AF.Tanh)
                nc.vector.tensor_mul(hT2[:, pair * N:(pair + 1) * N],
                                     sig[:, 2 * N:3 * N], tc_[:])

        # ---- readout head: feat = h[:, station 0, :] ----
        r1_ps = pp_agg.tile([H // 2, BL], fp32, tag="agg_ps")
        for b in range(BL):
            pair, q = divmod(b, 2)
            nc.tensor.matmul(
                r1_ps[:, b:b + 1],
                wr12_sb[q * H:(q + 1) * H, :],
                hT2[q * H:(q + 1) * H, pair * N:pair * N + 1],
                start=True, stop=True,
            )
        r1_sb = tmp_pool.tile([H // 2, BL], dt_data, tag="r1")
        nc.scalar.activation(r1_sb[:], r1_ps[:], AF.Relu, bias=br1_sb[:])
        r2_ps = pp_agg.tile([1, BL], fp32, tag="agg_ps")
        nc.tensor.matmul(r2_ps[:], wr2_sb[:], r1_sb[:], start=True, stop=True)
        out_sb = tmp_pool.tile([1, BL], fp32, tag="out")
        nc.scalar.activation(out_sb[:], r2_ps[:], AF.Identity, bias=br2_sb[:])
        nc.sync.dma_start(out_d[:], out_sb[:])

    nc.compile()
    return nc


def prep_inputs(pollution_seq, adj, Wg, bg, Wl, bl, Wr1, br1, Wr2, br2,
                t_steps: int = T, np_dt=np.float32):
    """Host-side prep: shard + relayout. Returns per-core in_maps list."""
    assert np.allclose(bl, 0.0), "kernel folds bl only for bl==0"
    f32 = np.float32

    adjT = np.ascontiguousarray(adj.T).astype(f32)          # [n, m]
    adjT_tiled = np.ascontiguousarray(
        adjT.reshape(KT, 128, N).transpose(1, 0, 2).reshape(128, KT * N)
    ).astype(np_dt)

    wgx = np.concatenate([Wg[:F], bg[None, :]], axis=0).astype(np_dt)   # [9, 64]
    wgh = Wg[F:].astype(f32)                                            # [64, 64]
    wgh2 = np.concatenate([wgh, wgh], axis=0).astype(np_dt)             # [128, 64]
    # gate order (i, f, g, o) -> (i, f, o, g)
    Wl_r = np.concatenate(
        [Wl[:, 0:H], Wl[:, H:2 * H], Wl[:, 3 * H:4 * H], Wl[:, 2 * H:3 * H]],
        axis=1).astype(f32)
    wl2 = np.concatenate([Wl_r, Wl_r], axis=0).astype(np_dt)            # [128, 256]
    wr12 = np.concatenate([Wr1, Wr1], axis=0).astype(np_dt)               # [128, 32]

    common = dict(
        adjT=adjT_tiled, wgx=wgx, wgh2=wgh2, wl2=wl2, wr12=wr12,
        br1=br1.reshape(H // 2, 1).astype(f32),
        wr2=Wr2.reshape(H // 2, 1).astype(np_dt),
        br2=br2.reshape(1, 1).astype(f32),
    )

    in_maps = []
    for i in range(NCORES):
        xc = pollution_seq[i * BL:(i + 1) * BL, :, :t_steps, :]   # [4, 512, t, 8]
        xT = np.ascontiguousarray(xc.transpose(3, 2, 0, 1))       # [8, t, 4, 512]
        xT = xT.reshape(F, t_steps, BN)
        ones = np.ones((1, t_steps, BN), dtype=f32)
        xT9 = np.concatenate([xT.astype(f32), ones], axis=0).astype(np_dt)
        m = dict(common)
        m["xT"] = np.ascontiguousarray(xT9)
        in_maps.append(m)
    return in_maps


_NC_CACHE = {}


def _get_program(t_steps, dt_data, dt_c):
    key = (t_steps, dt_data, dt_c)
    if key not in _NC_CACHE:
        _NC_CACHE[key] = build_program(t_steps, dt_data, dt_c)
    return _NC_CACHE[key]


def kernel(pollution_seq, adj, Wg, bg, Wl, bl, Wr1, br1, Wr2, br2,
           trace=False):
    if trace:
        _ensure_ntff_hook()
    if os.environ.get("GCLSTM_FP32"):
        dt_data, dt_c, np_dt = DT.float32, DT.float32, np.float32
    else:
        dt_data, dt_c, np_dt = DT.float16, DT.float32, np.float16
    nc = _get_program(T, dt_data, dt_c)
    in_maps = prep_inputs(pollution_seq, adj, Wg, bg, Wl, bl, Wr1, br1,
                          Wr2, br2, T, np_dt)
    res = run_bass_kernel_spmd(nc, in_maps, list(range(NCORES)), trace=trace)
    outs = [res.results[i]["out"].reshape(1, BL) for i in range(NCORES)]
    full = np.concatenate([o.T for o in outs], axis=0).astype(np.float32)  # [32,1]
    if trace:
        kernel.last_exec_time_ns = res.exec_time_ns
        kernel.last_results = res
    return full
